# revision 14
# baseline (speedup 1.0000x reference)
"""Trainium2 Bass kernel for the PlaneElement kinematic-wave step.

Contract: kernel(**inputs) takes the FULL (unsharded) numpy inputs and
returns the full output -- 4 scalars:
    (outflow_q, infil_rate_element, infil_depth_element, max_cfl)
as a float32 array of shape (4,).

Strategy (see git-less history in comments):
  - Shard the 4M-node axis contiguously across 8 NeuronCores; each core
    gets a [128, 4099] f32 tile: partition p holds 4096 owned nodes plus
    a 3-element stencil halo baked in on the host -> no device halo
    exchange, no collectives.
  - Device math per core (in SF = A/WID units), chunked for pipelining:
      SF      = relu(alpha*d + beta)      [ScalarE, fused sum accum]
      sum(d)                              [ScalarE copy, fused accum]
      minmod slope via clamp identity     [DVE]
      SFface  = SF + 0.5*minmod           [DVE scalar_tensor_tensor]
      flux'   = SFface * exp(2/3*(lnAs-lnwp) + ln(r*m))  [ScalarE+DVE]
      SFnext  = relu(SF - dflux')         [ScalarE]
      g2      = lnAs2 - lnwp2, reduce max [DVE]
    max(vel) = m*exp(2/3*max g2) on host (exp monotone);
    sum(infil) = sum(d) + C*b0 - sum(SF) (exact identity, halo terms
    subtracted on host).
  - Outlet discharge + the two inlet-boundary nodes computed exactly on
    the host in f64 from the device state / raw inputs.
"""

import math
import os
import tempfile

import numpy as np

N = 4_194_304
EPS = 1e-8
NCORES = 8
P = 128
F = 4096          # owned elements per partition
C = P * F         # owned elements per core
W = F + 3         # tile width incl. 3 halo columns

# chunk widths along the free dim (must sum to F); a small first chunk
# shortens the pipeline fill
CHUNKS = (512, 512, 1024, 1024, 1024)
BF16_STENCIL = False

_prog_cache = {}
_act_root = None


def _act_root_json():
    """Build an act-table root where natural_log_exp_and_others is the
    only/first set containing Ln and Exp, so walrus never thrashes
    ACT_TABLE_LOADs between per-function sets."""
    global _act_root
    if _act_root is not None:
        return _act_root
    import json
    from neuronxcc.driver.Job import Job
    from neuronxcc.driver.jobs.support.FindActInfo import findActInfoFile

    src = findActInfoFile(Job.getPackageDir(), "gen3")
    srcdir = os.path.dirname(src)
    d = json.load(open(src))
    sets = d["act_func_sets"]
    # Keep set order/indices identical to the canonical file (the NEFF
    # references sets by index and NRT resolves against its own copy);
    # just hide ln/exp from every other set so walrus's chooser lands on
    # the combined set for both.
    if any(s["name"] == "natural_log_exp_and_others" for s in sets):
        for s in sets:
            if s["name"] != "natural_log_exp_and_others":
                s["act"].pop("ln", None)
                s["act"].pop("exp", None)
    tmpd = tempfile.mkdtemp(prefix="actroot_")
    for fn in os.listdir(srcdir):
        if fn != os.path.basename(src):
            try:
                os.symlink(os.path.join(srcdir, fn), os.path.join(tmpd, fn))
            except OSError:
                pass
    dst = os.path.join(tmpd, os.path.basename(src))
    with open(dst, "w") as f:
        json.dump(d, f)
    _act_root = dst
    return dst


def _manning_q_np(A, WID, SS1, SS2, MAN, SL):
    h = A / WID
    wp = WID + h * (math.sqrt(1.0 + SS1 * SS1) + math.sqrt(1.0 + SS2 * SS2))
    A_safe = max(A, EPS)
    return A * (A_safe / wp) ** (2.0 / 3.0) * math.sqrt(SL) / MAN


def _build_program(consts, chunks, bf16):
    import concourse.bacc as bacc
    import concourse.mybir as mybir
    from concourse.tile import TileContext

    (alpha, beta, b0, sconst, wid, ln_rm) = consts
    nch = len(chunks)
    outc = 4 * nch + 1
    f32 = mybir.dt.float32
    bf = mybir.dt.bfloat16
    st_dt = bf if bf16 else f32
    Alu = mybir.AluOpType
    Act = mybir.ActivationFunctionType
    X = mybir.AxisListType.X

    nc = bacc.Bacc("TRN2", target_bir_lowering=False, debug=False,
                   num_devices=NCORES)
    d_in = nc.dram_tensor("d", [P, W], f32, kind="ExternalInput")
    o_out = nc.dram_tensor("out", [P, outc], f32, kind="ExternalOutput")

    with TileContext(nc) as tc:
        with tc.tile_pool(name="pool", bufs=2) as pool:
            # activation bias constants as Tile-managed [128,1] tiles
            b_beta = pool.tile([P, 1], f32, bufs=1)
            nc.vector.memset(b_beta[:], float(beta))
            b_eps = pool.tile([P, 1], f32, bufs=1)
            nc.vector.memset(b_eps[:], float(EPS))
            b_wid = pool.tile([P, 1], f32, bufs=1)
            nc.vector.memset(b_wid[:], float(wid))
            b_lnrm = pool.tile([P, 1], f32, bufs=1)
            nc.vector.memset(b_lnrm[:], float(ln_rm))

            out_tile = pool.tile([P, outc], f32, bufs=1)
            o = 0
            for c, cf in enumerate(chunks):
                L = cf + 3

                dd = pool.tile([P, L], f32, tag="dd")
                nc.sync.dma_start(out=dd[:], in_=d_in[:, o:o + L])

                # surface depth; fused window sums of SF and d
                SF = pool.tile([P, L], f32, tag="SF")
                nc.scalar.activation(SF[:], dd[:], Act.Relu,
                                     bias=b_beta[:], scale=alpha,
                                     accum_out=out_tile[:, c:c + 1])
                dsc = pool.tile([P, L], f32, tag="dsc")
                nc.scalar.activation(dsc[:], dd[:], Act.Copy,
                                     accum_out=out_tile[:, nch + c:
                                                        nch + c + 1])

                # MUSCL limiter: minmod(x,y) = clamp(y, min(x,0), max(x,0))
                dSF = pool.tile([P, L - 1], st_dt, tag="dSF")
                nc.vector.tensor_sub(dSF[:], SF[:, 1:L], SF[:, 0:L - 1])
                xm = pool.tile([P, L - 1], st_dt, tag="xm")
                nc.vector.tensor_scalar_min(xm[:], dSF[:], 0.0)
                xp = pool.tile([P, L - 1], st_dt, tag="xp")
                nc.vector.tensor_scalar_max(xp[:], dSF[:], 0.0)
                c1 = pool.tile([P, L - 2], st_dt, tag="c1")
                nc.vector.tensor_tensor(c1[:], dSF[:, 1:L - 1],
                                        xm[:, 0:L - 2], Alu.max)
                c2 = pool.tile([P, L - 2], st_dt, tag="c2")
                nc.vector.tensor_tensor(c2[:], c1[:], xp[:, 0:L - 2], Alu.min)
                SFf = pool.tile([P, L - 2], f32, tag="SFf")
                nc.vector.scalar_tensor_tensor(
                    SFf[:], c2[:], 0.5, SF[:, 1:L - 1], Alu.mult, Alu.add)

                # Manning flux on face states, in log space
                lnAs = pool.tile([P, L - 2], f32, tag="lnAs")
                nc.scalar.activation(lnAs[:], SFf[:], Act.Ln,
                                     bias=b_eps[:], scale=wid)
                lnwp = pool.tile([P, L - 2], f32, tag="lnwp")
                nc.scalar.activation(lnwp[:], SFf[:], Act.Ln,
                                     bias=b_wid[:], scale=sconst)
                g1 = pool.tile([P, L - 2], f32, tag="g1")
                nc.vector.tensor_sub(g1[:], lnAs[:], lnwp[:])
                pw = pool.tile([P, L - 2], f32, tag="pw")
                nc.scalar.activation(pw[:], g1[:], Act.Exp,
                                     bias=b_lnrm[:], scale=2.0 / 3.0)
                fx = pool.tile([P, L - 2], f32, tag="fx")
                nc.vector.tensor_mul(fx[:], SFf[:], pw[:])

                # conservative update
                fd = pool.tile([P, cf], f32, tag="fd")
                nc.vector.tensor_sub(fd[:], fx[:, 1:cf + 1], fx[:, 0:cf])
                s2 = pool.tile([P, cf], f32, tag="s2")
                nc.vector.tensor_sub(s2[:], SF[:, 2:2 + cf], fd[:])
                SFn = pool.tile([P, cf], f32, tag="SFn")
                nc.scalar.activation(SFn[:], s2[:], Act.Relu)

                # CFL: g2 = ln(A_safe) - ln(wp) on updated state, reduce max
                lnA2 = pool.tile([P, cf], f32, tag="lnA2")
                nc.scalar.activation(lnA2[:], SFn[:], Act.Ln,
                                     bias=b_eps[:], scale=wid)
                lnw2 = pool.tile([P, cf], f32, tag="lnw2")
                nc.scalar.activation(lnw2[:], SFn[:], Act.Ln,
                                     bias=b_wid[:], scale=sconst)
                g2 = pool.tile([P, cf], f32, tag="g2")
                nc.vector.tensor_sub(g2[:], lnA2[:], lnw2[:])
                nc.vector.tensor_reduce(
                    out_tile[:, 2 * nch + c:2 * nch + c + 1], g2[:, 2:cf],
                    X, Alu.max)
                nc.vector.tensor_reduce(
                    out_tile[:, 3 * nch + c:3 * nch + c + 1], g2[:, 0:2],
                    X, Alu.max)

                if c == nch - 1:
                    nc.vector.tensor_copy(out_tile[:, 4 * nch:4 * nch + 1],
                                          SFn[:, cf - 1:cf])
                o += cf

            nc.sync.dma_start(out=o_out[:, :], in_=out_tile[:])

    nc.compile()
    return nc


def _run_device(shards, consts, chunks, bf16, trace=False):
    from concourse.bass_utils import run_bass_kernel_spmd

    os.environ["BASS_ACT_ROOT_JSON_PATH"] = _act_root_json()
    key = (tuple(consts), tuple(chunks), bf16)
    if key not in _prog_cache:
        _prog_cache[key] = _build_program(consts, chunks, bf16)
    nc = _prog_cache[key]
    in_maps = [{"d": shards[i]} for i in range(NCORES)]
    res = run_bass_kernel_spmd(nc, in_maps, core_ids=list(range(NCORES)),
                               trace=trace)
    return res


def kernel(depth, rain_rate, dt, cum_rain, theta_current, F_cumulative,
           WID, SS1, SS2, MAN, SL, dx, Ks, psi, theta_s, _trace=False,
           _return_results=False, _chunks=CHUNKS, _bf16=BF16_STENCIL):
    depth = np.asarray(depth, np.float32)
    rain_rate = float(rain_rate)
    dt = float(dt)
    theta_current = float(theta_current)
    F_cumulative = float(F_cumulative)
    WID = float(WID)
    SS1 = float(SS1)
    SS2 = float(SS2)
    MAN = float(MAN)
    SL = float(SL)
    dx = float(dx)
    Ks = float(Ks)
    psi = float(psi)
    theta_s = float(theta_s)

    # host-folded scalar coefficients (f64)
    dtheta = max(theta_s - theta_current, 0.0)
    F_safe = max(F_cumulative, 1e-6)
    a1 = Ks * dt / F_safe                       # fp*dt = a0 + a1*d
    a0 = Ks * dt * (1.0 + psi * dtheta / F_safe)
    b0 = rain_rate * dt                         # avail = d + b0
    alpha = 1.0 - a1                            # surf = relu(alpha*d + beta)
    beta = b0 - a0
    sconst = math.sqrt(1.0 + SS1 * SS1) + math.sqrt(1.0 + SS2 * SS2)
    m = math.sqrt(SL) / MAN
    r = dt / dx
    # In SF = A/WID units: SF_next = relu(SF - (f_i - f_{i-1})) with
    #   f = (r/WID)*q(A_face) = r*m*SFface*ratio^(2/3),
    #   ratio = max(WID*SFface, EPS)/(WID + sconst*SFface)
    # lnAs = ln(WID*SFface + EPS), lnwp = ln(WID + sconst*SFface),
    # exp bias = ln(r*m).
    ln_rm = math.log(max(r * m, 1e-38))
    consts = (alpha, beta, b0, sconst, WID, ln_rm)

    # --- host shard prep: [128, 4099] per core with baked halo ---
    padded = np.empty(N + 3, np.float32)
    padded[2:2 + N] = depth
    padded[0:2] = 0.0          # left ghosts (nodes 0,1 host-corrected)
    padded[N + 2] = depth[-1]  # right ghost replicates -> slope[N-1] = 0
    shards = []
    for k in range(NCORES):
        base = padded[k * C:k * C + C + 3]
        sh = np.lib.stride_tricks.as_strided(
            base, shape=(P, W), strides=(F * 4, 4)).copy()
        shards.append(np.ascontiguousarray(sh))

    res = _run_device(shards, consts, _chunks, _bf16, trace=_trace)
    outs = [res.results[i]["out"] for i in range(NCORES)]

    nch = len(_chunks)

    # --- host combine ---
    # halo columns per chunk: local j in {o, o+1, o+cf+2}
    halo_j = []
    o = 0
    for cf in _chunks:
        halo_j += [o, o + 1, o + cf + 2]
        o += cf
    halo_j = np.array(halo_j)
    sum_SF = np.float64(0.0)
    sum_d = np.float64(0.0)
    for k in range(NCORES):
        sum_SF += np.sum(outs[k][:, 0:nch].astype(np.float64))
        sum_d += np.sum(outs[k][:, nch:2 * nch].astype(np.float64))
        dh = shards[k][:, halo_j].astype(np.float64)
        sum_d -= dh.sum()
        sum_SF -= np.maximum(alpha * dh + beta, 0.0).sum()
    sum_t = sum_d + N * b0 - sum_SF
    infil_depth = sum_t / N
    infil_rate = infil_depth / dt

    g2max = -np.inf
    for k in range(NCORES):
        g2max = max(g2max, float(outs[k][:, 2 * nch:3 * nch].max()))
        edge = outs[k][:, 3 * nch:4 * nch].astype(np.float64).copy()
        if k == 0:
            edge[0, 0] = -np.inf  # polluted inlet nodes 0,1
        g2max = max(g2max, float(edge.max()))
    max_vel = m * math.exp((2.0 / 3.0) * g2max) if m > 0 else 0.0

    # exact inlet nodes 0 and 1 on host (f64), matching reference BCs
    d0, d1, d2 = (float(depth[0]), float(depth[1]), float(depth[2]))

    def _surf(d):
        t = min(a0 + a1 * d, d + b0)
        return max(d + b0 - t, 0.0)

    A0, A1, A2 = (WID * _surf(d0), WID * _surf(d1), WID * _surf(d2))
    # slope[0] = 0; slope[1] = minmod(A1-A0, A2-A1)
    x, y = A1 - A0, A2 - A1
    mm1 = min(max(y, min(x, 0.0)), max(x, 0.0))
    Af0 = A0
    Af1 = A1 + 0.5 * mm1
    q0 = _manning_q_np(Af0, WID, SS1, SS2, MAN, SL)
    q1 = _manning_q_np(Af1, WID, SS1, SS2, MAN, SL)
    An0 = max(A0 - r * (q0 - 0.0), 0.0)
    An1 = max(A1 - r * (q1 - q0), 0.0)
    for An in (An0, An1):
        Q = _manning_q_np(An, WID, SS1, SS2, MAN, SL)
        max_vel = max(max_vel, Q / max(An, EPS))

    max_cfl = max_vel * dt / dx

    # outlet discharge from the device's last updated state
    sfl = float(outs[NCORES - 1][P - 1, 4 * nch])
    A_last = WID * sfl
    outflow_q = _manning_q_np(A_last, WID, SS1, SS2, MAN, SL)

    out = np.array([outflow_q, infil_rate, infil_depth, max_cfl], np.float32)
    if _return_results:
        return out, res
    return out


# revision 17
# speedup vs baseline: 1.1851x; 1.1851x over previous
"""Trainium2 Bass kernel for the PlaneElement kinematic-wave step.

Contract: kernel(**inputs) takes the FULL (unsharded) numpy inputs and
returns the full output -- 4 scalars:
    (outflow_q, infil_rate_element, infil_depth_element, max_cfl)
as a float32 array of shape (4,).

Strategy (see git-less history in comments):
  - Shard the 4M-node axis contiguously across 8 NeuronCores; each core
    gets a [128, 4099] f32 tile: partition p holds 4096 owned nodes plus
    a 3-element stencil halo baked in on the host -> no device halo
    exchange, no collectives.
  - Device math per core (in SF = A/WID units), chunked for pipelining:
      SF      = relu(alpha*d + beta)      [ScalarE, fused sum accum]
      sum(d)                              [ScalarE copy, fused accum]
      minmod slope via clamp identity     [DVE]
      SFface  = SF + 0.5*minmod           [DVE scalar_tensor_tensor]
      flux'   = SFface * exp(2/3*(lnAs-lnwp) + ln(r*m))  [ScalarE+DVE]
      SFnext  = relu(SF - dflux')         [ScalarE]
      g2      = lnAs2 - lnwp2, reduce max [DVE]
    max(vel) = m*exp(2/3*max g2) on host (exp monotone);
    sum(infil) = sum(d) + C*b0 - sum(SF) (exact identity, halo terms
    subtracted on host).
  - Outlet discharge + the two inlet-boundary nodes computed exactly on
    the host in f64 from the device state / raw inputs.
"""

import math
import os
import tempfile

import numpy as np

N = 4_194_304
EPS = 1e-8
NCORES = 8
P = 128
F = 4096          # owned elements per partition
C = P * F         # owned elements per core
W = F + 3         # tile width incl. 3 halo columns

# chunk widths along the free dim (must sum to F)
CHUNKS = (1024, 1024, 1024, 1024)
BF16_STENCIL = False

_prog_cache = {}
_act_patch_done = False


def _patch_act_tables():
    """Force Bacc's act-table-load placement to use the single
    natural_log_exp_and_others set for every activation we emit (Relu,
    Copy, Identity, Ln, Exp). Set order/indices are preserved, only the
    per-set membership shown to the placement pass is filtered, so the
    emitted act_func_set_id still matches the canonical act_info.json
    that walrus and NRT resolve against. Result: one ACT_TABLE_LOAD for
    the whole kernel instead of per-transition thrash."""
    global _act_patch_done
    if _act_patch_done:
        return
    import concourse.hw_specs as hw_specs
    import concourse.bacc as bacc_mod
    import concourse.mybir as mybir

    Act = mybir.ActivationFunctionType
    mine = {Act.Relu, Act.Copy, Act.Identity, Act.Ln, Act.Exp}
    orig = hw_specs.get_activation_tables

    def patched(module_arch):
        tabs = orig(module_arch)
        if "natural_log_exp_and_others" not in tabs:
            return tabs
        out = {}
        for name, funcs in tabs.items():
            if name != "natural_log_exp_and_others":
                funcs = funcs - mine
            out[name] = funcs
        return out

    hw_specs.get_activation_tables = patched
    bacc_mod.get_activation_tables = patched
    _act_patch_done = True


def _manning_q_np(A, WID, SS1, SS2, MAN, SL):
    h = A / WID
    wp = WID + h * (math.sqrt(1.0 + SS1 * SS1) + math.sqrt(1.0 + SS2 * SS2))
    A_safe = max(A, EPS)
    return A * (A_safe / wp) ** (2.0 / 3.0) * math.sqrt(SL) / MAN


def _build_program(consts, chunks, bf16):
    import concourse.bacc as bacc
    import concourse.mybir as mybir
    from concourse.tile import TileContext

    (alpha, beta, b0, sconst, wid, ln_rm) = consts
    nch = len(chunks)
    outc = 4 * nch + 1
    f32 = mybir.dt.float32
    bf = mybir.dt.bfloat16
    st_dt = bf if bf16 else f32
    Alu = mybir.AluOpType
    Act = mybir.ActivationFunctionType
    X = mybir.AxisListType.X

    nc = bacc.Bacc("TRN2", target_bir_lowering=False, debug=False,
                   num_devices=NCORES)
    d_in = nc.dram_tensor("d", [P, W], f32, kind="ExternalInput")
    o_out = nc.dram_tensor("out", [P, outc], f32, kind="ExternalOutput")

    with TileContext(nc) as tc:
        with tc.tile_pool(name="pool", bufs=2) as pool:
            # activation bias constants as Tile-managed [128,1] tiles
            b_beta = pool.tile([P, 1], f32, bufs=1)
            nc.vector.memset(b_beta[:], float(beta))
            b_eps = pool.tile([P, 1], f32, bufs=1)
            nc.vector.memset(b_eps[:], float(EPS))
            b_wid = pool.tile([P, 1], f32, bufs=1)
            nc.vector.memset(b_wid[:], float(wid))
            b_lnrm = pool.tile([P, 1], f32, bufs=1)
            nc.vector.memset(b_lnrm[:], float(ln_rm))

            out_tile = pool.tile([P, outc], f32, bufs=1)
            o = 0
            for c, cf in enumerate(chunks):
                L = cf + 3

                dd = pool.tile([P, L], f32, tag="dd")
                nc.sync.dma_start(out=dd[:], in_=d_in[:, o:o + L])

                # surface depth; fused window sums of SF and d
                SF = pool.tile([P, L], f32, tag="SF")
                nc.scalar.activation(SF[:], dd[:], Act.Relu,
                                     bias=b_beta[:], scale=alpha,
                                     accum_out=out_tile[:, c:c + 1])
                dsc = pool.tile([P, L], f32, tag="dsc")
                nc.scalar.activation(dsc[:], dd[:], Act.Copy,
                                     accum_out=out_tile[:, nch + c:
                                                        nch + c + 1])

                # MUSCL limiter: minmod(x,y) = clamp(y, min(x,0), max(x,0))
                dSF = pool.tile([P, L - 1], st_dt, tag="dSF")
                nc.vector.tensor_sub(dSF[:], SF[:, 1:L], SF[:, 0:L - 1])
                xm = pool.tile([P, L - 1], st_dt, tag="xm")
                nc.vector.tensor_scalar_min(xm[:], dSF[:], 0.0)
                xp = pool.tile([P, L - 1], st_dt, tag="xp")
                nc.vector.tensor_scalar_max(xp[:], dSF[:], 0.0)
                c1 = pool.tile([P, L - 2], st_dt, tag="c1")
                nc.vector.tensor_tensor(c1[:], dSF[:, 1:L - 1],
                                        xm[:, 0:L - 2], Alu.max)
                c2 = pool.tile([P, L - 2], st_dt, tag="c2")
                nc.vector.tensor_tensor(c2[:], c1[:], xp[:, 0:L - 2], Alu.min)
                SFf = pool.tile([P, L - 2], f32, tag="SFf")
                nc.vector.scalar_tensor_tensor(
                    SFf[:], c2[:], 0.5, SF[:, 1:L - 1], Alu.mult, Alu.add)

                # Manning flux on face states, in log space
                lnAs = pool.tile([P, L - 2], f32, tag="lnAs")
                nc.scalar.activation(lnAs[:], SFf[:], Act.Ln,
                                     bias=b_eps[:], scale=wid)
                lnwp = pool.tile([P, L - 2], f32, tag="lnwp")
                nc.scalar.activation(lnwp[:], SFf[:], Act.Ln,
                                     bias=b_wid[:], scale=sconst)
                g1 = pool.tile([P, L - 2], f32, tag="g1")
                nc.vector.tensor_sub(g1[:], lnAs[:], lnwp[:])
                pw = pool.tile([P, L - 2], f32, tag="pw")
                nc.scalar.activation(pw[:], g1[:], Act.Exp,
                                     bias=b_lnrm[:], scale=2.0 / 3.0)
                fx = pool.tile([P, L - 2], f32, tag="fx")
                nc.vector.tensor_mul(fx[:], SFf[:], pw[:])

                # conservative update
                fd = pool.tile([P, cf], f32, tag="fd")
                nc.vector.tensor_sub(fd[:], fx[:, 1:cf + 1], fx[:, 0:cf])
                s2 = pool.tile([P, cf], f32, tag="s2")
                nc.vector.tensor_sub(s2[:], SF[:, 2:2 + cf], fd[:])
                SFn = pool.tile([P, cf], f32, tag="SFn")
                nc.scalar.activation(SFn[:], s2[:], Act.Relu)

                # CFL: g2 = ln(A_safe) - ln(wp) on updated state, reduce max
                lnA2 = pool.tile([P, cf], f32, tag="lnA2")
                nc.scalar.activation(lnA2[:], SFn[:], Act.Ln,
                                     bias=b_eps[:], scale=wid)
                lnw2 = pool.tile([P, cf], f32, tag="lnw2")
                nc.scalar.activation(lnw2[:], SFn[:], Act.Ln,
                                     bias=b_wid[:], scale=sconst)
                g2 = pool.tile([P, cf], f32, tag="g2")
                nc.vector.tensor_sub(g2[:], lnA2[:], lnw2[:])
                nc.vector.tensor_reduce(
                    out_tile[:, 2 * nch + c:2 * nch + c + 1], g2[:, 2:cf],
                    X, Alu.max)
                nc.vector.tensor_reduce(
                    out_tile[:, 3 * nch + c:3 * nch + c + 1], g2[:, 0:2],
                    X, Alu.max)

                if c == nch - 1:
                    nc.vector.tensor_copy(out_tile[:, 4 * nch:4 * nch + 1],
                                          SFn[:, cf - 1:cf])
                o += cf

            nc.sync.dma_start(out=o_out[:, :], in_=out_tile[:])

    nc.compile()
    return nc


def _run_device(shards, consts, chunks, bf16, trace=False):
    from concourse.bass_utils import run_bass_kernel_spmd

    _patch_act_tables()
    key = (tuple(consts), tuple(chunks), bf16)
    if key not in _prog_cache:
        _prog_cache[key] = _build_program(consts, chunks, bf16)
    nc = _prog_cache[key]
    in_maps = [{"d": shards[i]} for i in range(NCORES)]
    res = run_bass_kernel_spmd(nc, in_maps, core_ids=list(range(NCORES)),
                               trace=trace)
    return res


def kernel(depth, rain_rate, dt, cum_rain, theta_current, F_cumulative,
           WID, SS1, SS2, MAN, SL, dx, Ks, psi, theta_s, _trace=False,
           _return_results=False, _chunks=CHUNKS, _bf16=BF16_STENCIL):
    depth = np.asarray(depth, np.float32)
    rain_rate = float(rain_rate)
    dt = float(dt)
    theta_current = float(theta_current)
    F_cumulative = float(F_cumulative)
    WID = float(WID)
    SS1 = float(SS1)
    SS2 = float(SS2)
    MAN = float(MAN)
    SL = float(SL)
    dx = float(dx)
    Ks = float(Ks)
    psi = float(psi)
    theta_s = float(theta_s)

    # host-folded scalar coefficients (f64)
    dtheta = max(theta_s - theta_current, 0.0)
    F_safe = max(F_cumulative, 1e-6)
    a1 = Ks * dt / F_safe                       # fp*dt = a0 + a1*d
    a0 = Ks * dt * (1.0 + psi * dtheta / F_safe)
    b0 = rain_rate * dt                         # avail = d + b0
    alpha = 1.0 - a1                            # surf = relu(alpha*d + beta)
    beta = b0 - a0
    sconst = math.sqrt(1.0 + SS1 * SS1) + math.sqrt(1.0 + SS2 * SS2)
    m = math.sqrt(SL) / MAN
    r = dt / dx
    # In SF = A/WID units: SF_next = relu(SF - (f_i - f_{i-1})) with
    #   f = (r/WID)*q(A_face) = r*m*SFface*ratio^(2/3),
    #   ratio = max(WID*SFface, EPS)/(WID + sconst*SFface)
    # lnAs = ln(WID*SFface + EPS), lnwp = ln(WID + sconst*SFface),
    # exp bias = ln(r*m).
    ln_rm = math.log(max(r * m, 1e-38))
    consts = (alpha, beta, b0, sconst, WID, ln_rm)

    # --- host shard prep: [128, 4099] per core with baked halo ---
    padded = np.empty(N + 3, np.float32)
    padded[2:2 + N] = depth
    padded[0:2] = 0.0          # left ghosts (nodes 0,1 host-corrected)
    padded[N + 2] = depth[-1]  # right ghost replicates -> slope[N-1] = 0
    shards = []
    for k in range(NCORES):
        base = padded[k * C:k * C + C + 3]
        sh = np.lib.stride_tricks.as_strided(
            base, shape=(P, W), strides=(F * 4, 4)).copy()
        shards.append(np.ascontiguousarray(sh))

    res = _run_device(shards, consts, _chunks, _bf16, trace=_trace)
    outs = [res.results[i]["out"] for i in range(NCORES)]

    nch = len(_chunks)

    # --- host combine ---
    # halo columns per chunk: local j in {o, o+1, o+cf+2}
    halo_j = []
    o = 0
    for cf in _chunks:
        halo_j += [o, o + 1, o + cf + 2]
        o += cf
    halo_j = np.array(halo_j)
    sum_SF = np.float64(0.0)
    sum_d = np.float64(0.0)
    for k in range(NCORES):
        sum_SF += np.sum(outs[k][:, 0:nch].astype(np.float64))
        sum_d += np.sum(outs[k][:, nch:2 * nch].astype(np.float64))
        dh = shards[k][:, halo_j].astype(np.float64)
        sum_d -= dh.sum()
        sum_SF -= np.maximum(alpha * dh + beta, 0.0).sum()
    sum_t = sum_d + N * b0 - sum_SF
    infil_depth = sum_t / N
    infil_rate = infil_depth / dt

    g2max = -np.inf
    for k in range(NCORES):
        g2max = max(g2max, float(outs[k][:, 2 * nch:3 * nch].max()))
        edge = outs[k][:, 3 * nch:4 * nch].astype(np.float64).copy()
        if k == 0:
            edge[0, 0] = -np.inf  # polluted inlet nodes 0,1
        g2max = max(g2max, float(edge.max()))
    max_vel = m * math.exp((2.0 / 3.0) * g2max) if m > 0 else 0.0

    # exact inlet nodes 0 and 1 on host (f64), matching reference BCs
    d0, d1, d2 = (float(depth[0]), float(depth[1]), float(depth[2]))

    def _surf(d):
        t = min(a0 + a1 * d, d + b0)
        return max(d + b0 - t, 0.0)

    A0, A1, A2 = (WID * _surf(d0), WID * _surf(d1), WID * _surf(d2))
    # slope[0] = 0; slope[1] = minmod(A1-A0, A2-A1)
    x, y = A1 - A0, A2 - A1
    mm1 = min(max(y, min(x, 0.0)), max(x, 0.0))
    Af0 = A0
    Af1 = A1 + 0.5 * mm1
    q0 = _manning_q_np(Af0, WID, SS1, SS2, MAN, SL)
    q1 = _manning_q_np(Af1, WID, SS1, SS2, MAN, SL)
    An0 = max(A0 - r * (q0 - 0.0), 0.0)
    An1 = max(A1 - r * (q1 - q0), 0.0)
    for An in (An0, An1):
        Q = _manning_q_np(An, WID, SS1, SS2, MAN, SL)
        max_vel = max(max_vel, Q / max(An, EPS))

    max_cfl = max_vel * dt / dx

    # outlet discharge from the device's last updated state
    sfl = float(outs[NCORES - 1][P - 1, 4 * nch])
    A_last = WID * sfl
    outflow_q = _manning_q_np(A_last, WID, SS1, SS2, MAN, SL)

    out = np.array([outflow_q, infil_rate, infil_depth, max_cfl], np.float32)
    if _return_results:
        return out, res
    return out


# revision 21
# speedup vs baseline: 1.2520x; 1.0565x over previous
"""Trainium2 Bass kernel for the PlaneElement kinematic-wave step.

Contract: kernel(**inputs) takes the FULL (unsharded) numpy inputs and
returns the full output -- 4 scalars:
    (outflow_q, infil_rate_element, infil_depth_element, max_cfl)
as a float32 array of shape (4,).

Strategy (see git-less history in comments):
  - Shard the 4M-node axis contiguously across 8 NeuronCores; each core
    gets a [128, 4099] f32 tile: partition p holds 4096 owned nodes plus
    a 3-element stencil halo baked in on the host -> no device halo
    exchange, no collectives.
  - Device math per core (in SF = A/WID units), chunked for pipelining:
      SF      = relu(alpha*d + beta)      [ScalarE, fused sum accum]
      sum(d)                              [ScalarE copy, fused accum]
      minmod slope via clamp identity     [DVE]
      SFface  = SF + 0.5*minmod           [DVE scalar_tensor_tensor]
      flux'   = SFface * exp(2/3*(lnAs-lnwp) + ln(r*m))  [ScalarE+DVE]
      SFnext  = relu(SF - dflux')         [ScalarE]
      g2      = lnAs2 - lnwp2, reduce max [DVE]
    max(vel) = m*exp(2/3*max g2) on host (exp monotone);
    sum(infil) = sum(d) + C*b0 - sum(SF) (exact identity, halo terms
    subtracted on host).
  - Outlet discharge + the two inlet-boundary nodes computed exactly on
    the host in f64 from the device state / raw inputs.
"""

import math
import os
import tempfile

import numpy as np

N = 4_194_304
EPS = 1e-8
NCORES = 8
P = 128
F = 4096          # owned elements per partition
C = P * F         # owned elements per core
W = F + 3         # tile width incl. 3 halo columns

# chunk widths along the free dim (must sum to F); small first chunk
# shortens the pipeline fill
CHUNKS = (256, 960, 960, 960, 960)
BF16_STENCIL = False

_prog_cache = {}
_act_patch_done = False


def _patch_act_tables():
    """Force Bacc's act-table-load placement to use the single
    natural_log_exp_and_others set for every activation we emit (Relu,
    Copy, Identity, Ln, Exp). Set order/indices are preserved, only the
    per-set membership shown to the placement pass is filtered, so the
    emitted act_func_set_id still matches the canonical act_info.json
    that walrus and NRT resolve against. Result: one ACT_TABLE_LOAD for
    the whole kernel instead of per-transition thrash."""
    global _act_patch_done
    if _act_patch_done:
        return
    import concourse.hw_specs as hw_specs
    import concourse.bacc as bacc_mod
    import concourse.mybir as mybir

    Act = mybir.ActivationFunctionType
    mine = {Act.Relu, Act.Copy, Act.Identity, Act.Ln, Act.Exp}
    orig = hw_specs.get_activation_tables

    def patched(module_arch):
        tabs = orig(module_arch)
        if "natural_log_exp_and_others" not in tabs:
            return tabs
        out = {}
        for name, funcs in tabs.items():
            if name != "natural_log_exp_and_others":
                funcs = funcs - mine
            out[name] = funcs
        return out

    hw_specs.get_activation_tables = patched
    bacc_mod.get_activation_tables = patched
    _act_patch_done = True


def _manning_q_np(A, WID, SS1, SS2, MAN, SL):
    h = A / WID
    wp = WID + h * (math.sqrt(1.0 + SS1 * SS1) + math.sqrt(1.0 + SS2 * SS2))
    A_safe = max(A, EPS)
    return A * (A_safe / wp) ** (2.0 / 3.0) * math.sqrt(SL) / MAN


def _build_program(consts, chunks, bf16):
    import concourse.bacc as bacc
    import concourse.mybir as mybir
    from concourse.tile import TileContext

    (alpha, beta, b0, sconst, wid, ln_rm) = consts
    nch = len(chunks)
    outc = 4 * nch + 1
    f32 = mybir.dt.float32
    bf = mybir.dt.bfloat16
    st_dt = bf if bf16 else f32
    Alu = mybir.AluOpType
    Act = mybir.ActivationFunctionType
    X = mybir.AxisListType.X

    nc = bacc.Bacc("TRN2", target_bir_lowering=False, debug=False,
                   num_devices=NCORES)
    d_in = nc.dram_tensor("d", [P, W], f32, kind="ExternalInput")
    o_out = nc.dram_tensor("out", [P, outc], f32, kind="ExternalOutput")

    with TileContext(nc) as tc:
        with tc.tile_pool(name="pool", bufs=2) as pool:
            # activation bias constants as Tile-managed [128,1] tiles
            b_beta = pool.tile([P, 1], f32, bufs=1)
            nc.vector.memset(b_beta[:], float(beta))
            b_eps = pool.tile([P, 1], f32, bufs=1)
            nc.vector.memset(b_eps[:], float(EPS))
            b_wid = pool.tile([P, 1], f32, bufs=1)
            nc.vector.memset(b_wid[:], float(wid))
            b_lnrm = pool.tile([P, 1], f32, bufs=1)
            nc.vector.memset(b_lnrm[:], float(ln_rm))

            out_tile = pool.tile([P, outc], f32, bufs=1)
            o = 0
            for c, cf in enumerate(chunks):
                L = cf + 3

                dd = pool.tile([P, L], f32, tag="dd")
                nc.sync.dma_start(out=dd[:], in_=d_in[:, o:o + L])

                # surface depth; fused window sums of SF and d
                SF = pool.tile([P, L], f32, tag="SF")
                nc.scalar.activation(SF[:], dd[:], Act.Relu,
                                     bias=b_beta[:], scale=alpha,
                                     accum_out=out_tile[:, c:c + 1])
                dsc = pool.tile([P, L], f32, tag="dsc")
                nc.scalar.activation(dsc[:], dd[:], Act.Copy,
                                     accum_out=out_tile[:, nch + c:
                                                        nch + c + 1])

                # MUSCL limiter: minmod(x,y) = clamp(y, min(x,0), max(x,0))
                dSF = pool.tile([P, L - 1], st_dt, tag="dSF")
                nc.vector.tensor_sub(dSF[:], SF[:, 1:L], SF[:, 0:L - 1])
                xm = pool.tile([P, L - 1], st_dt, tag="xm")
                nc.vector.tensor_scalar_min(xm[:], dSF[:], 0.0)
                xp = pool.tile([P, L - 1], st_dt, tag="xp")
                nc.vector.tensor_scalar_max(xp[:], dSF[:], 0.0)
                c1 = pool.tile([P, L - 2], st_dt, tag="c1")
                nc.vector.tensor_tensor(c1[:], dSF[:, 1:L - 1],
                                        xm[:, 0:L - 2], Alu.max)
                c2 = pool.tile([P, L - 2], st_dt, tag="c2")
                nc.vector.tensor_tensor(c2[:], c1[:], xp[:, 0:L - 2], Alu.min)
                SFf = pool.tile([P, L - 2], st_dt, tag="SFf")
                nc.vector.scalar_tensor_tensor(
                    SFf[:], c2[:], 0.5, SF[:, 1:L - 1], Alu.mult, Alu.add)

                # Manning flux on face states, in log space
                lnAs = pool.tile([P, L - 2], f32, tag="lnAs")
                nc.scalar.activation(lnAs[:], SFf[:], Act.Ln,
                                     bias=b_eps[:], scale=wid)
                lnwp = pool.tile([P, L - 2], f32, tag="lnwp")
                nc.scalar.activation(lnwp[:], SFf[:], Act.Ln,
                                     bias=b_wid[:], scale=sconst)
                g1 = pool.tile([P, L - 2], f32, tag="g1")
                nc.vector.tensor_sub(g1[:], lnAs[:], lnwp[:])
                pw = pool.tile([P, L - 2], st_dt, tag="pw")
                nc.scalar.activation(pw[:], g1[:], Act.Exp,
                                     bias=b_lnrm[:], scale=2.0 / 3.0)
                fx = pool.tile([P, L - 2], st_dt, tag="fx")
                nc.vector.tensor_mul(fx[:], SFf[:], pw[:])

                # conservative update; s2 = SF - dflux (relu deferred to
                # host: max(vel) = vel(relu(max s2)) since vel and relu
                # are monotone)
                fd = pool.tile([P, cf], st_dt, tag="fd")
                nc.vector.tensor_sub(fd[:], fx[:, 1:cf + 1], fx[:, 0:cf])
                s2 = pool.tile([P, cf], f32, tag="s2")
                nc.vector.tensor_sub(s2[:], SF[:, 2:2 + cf], fd[:])
                nc.vector.tensor_reduce(
                    out_tile[:, 2 * nch + c:2 * nch + c + 1], s2[:, 2:cf],
                    X, Alu.max)
                nc.vector.tensor_reduce(
                    out_tile[:, 3 * nch + c:3 * nch + c + 1], s2[:, 0:2],
                    X, Alu.max)

                if c == nch - 1:
                    nc.vector.tensor_copy(out_tile[:, 4 * nch:4 * nch + 1],
                                          s2[:, cf - 1:cf])
                o += cf

            nc.sync.dma_start(out=o_out[:, :], in_=out_tile[:])

    nc.compile()
    return nc


def _run_device(shards, consts, chunks, bf16, trace=False):
    from concourse.bass_utils import run_bass_kernel_spmd

    _patch_act_tables()
    key = (tuple(consts), tuple(chunks), bf16)
    if key not in _prog_cache:
        _prog_cache[key] = _build_program(consts, chunks, bf16)
    nc = _prog_cache[key]
    in_maps = [{"d": shards[i]} for i in range(NCORES)]
    res = run_bass_kernel_spmd(nc, in_maps, core_ids=list(range(NCORES)),
                               trace=trace)
    return res


def kernel(depth, rain_rate, dt, cum_rain, theta_current, F_cumulative,
           WID, SS1, SS2, MAN, SL, dx, Ks, psi, theta_s, _trace=False,
           _return_results=False, _chunks=CHUNKS, _bf16=BF16_STENCIL):
    depth = np.asarray(depth, np.float32)
    rain_rate = float(rain_rate)
    dt = float(dt)
    theta_current = float(theta_current)
    F_cumulative = float(F_cumulative)
    WID = float(WID)
    SS1 = float(SS1)
    SS2 = float(SS2)
    MAN = float(MAN)
    SL = float(SL)
    dx = float(dx)
    Ks = float(Ks)
    psi = float(psi)
    theta_s = float(theta_s)

    # host-folded scalar coefficients (f64)
    dtheta = max(theta_s - theta_current, 0.0)
    F_safe = max(F_cumulative, 1e-6)
    a1 = Ks * dt / F_safe                       # fp*dt = a0 + a1*d
    a0 = Ks * dt * (1.0 + psi * dtheta / F_safe)
    b0 = rain_rate * dt                         # avail = d + b0
    alpha = 1.0 - a1                            # surf = relu(alpha*d + beta)
    beta = b0 - a0
    sconst = math.sqrt(1.0 + SS1 * SS1) + math.sqrt(1.0 + SS2 * SS2)
    m = math.sqrt(SL) / MAN
    r = dt / dx
    # In SF = A/WID units: SF_next = relu(SF - (f_i - f_{i-1})) with
    #   f = (r/WID)*q(A_face) = r*m*SFface*ratio^(2/3),
    #   ratio = max(WID*SFface, EPS)/(WID + sconst*SFface)
    # lnAs = ln(WID*SFface + EPS), lnwp = ln(WID + sconst*SFface),
    # exp bias = ln(r*m).
    ln_rm = math.log(max(r * m, 1e-38))
    consts = (alpha, beta, b0, sconst, WID, ln_rm)

    # --- host shard prep: [128, 4099] per core with baked halo ---
    padded = np.empty(N + 3, np.float32)
    padded[2:2 + N] = depth
    padded[0:2] = 0.0          # left ghosts (nodes 0,1 host-corrected)
    padded[N + 2] = depth[-1]  # right ghost replicates -> slope[N-1] = 0
    shards = []
    for k in range(NCORES):
        base = padded[k * C:k * C + C + 3]
        sh = np.lib.stride_tricks.as_strided(
            base, shape=(P, W), strides=(F * 4, 4)).copy()
        shards.append(np.ascontiguousarray(sh))

    res = _run_device(shards, consts, _chunks, _bf16, trace=_trace)
    outs = [res.results[i]["out"] for i in range(NCORES)]

    nch = len(_chunks)

    # --- host combine ---
    # halo columns per chunk: local j in {o, o+1, o+cf+2}
    halo_j = []
    o = 0
    for cf in _chunks:
        halo_j += [o, o + 1, o + cf + 2]
        o += cf
    halo_j = np.array(halo_j)
    sum_SF = np.float64(0.0)
    sum_d = np.float64(0.0)
    for k in range(NCORES):
        sum_SF += np.sum(outs[k][:, 0:nch].astype(np.float64))
        sum_d += np.sum(outs[k][:, nch:2 * nch].astype(np.float64))
        dh = shards[k][:, halo_j].astype(np.float64)
        sum_d -= dh.sum()
        sum_SF -= np.maximum(alpha * dh + beta, 0.0).sum()
    sum_t = sum_d + N * b0 - sum_SF
    infil_depth = sum_t / N
    infil_rate = infil_depth / dt

    s2max = -np.inf
    for k in range(NCORES):
        s2max = max(s2max, float(outs[k][:, 2 * nch:3 * nch].max()))
        edge = outs[k][:, 3 * nch:4 * nch].astype(np.float64).copy()
        if k == 0:
            edge[0, 0] = -np.inf  # polluted inlet nodes 0,1
        s2max = max(s2max, float(edge.max()))
    # vel is monotone in SF_next, so the max velocity comes from the max
    # updated state, evaluated exactly here
    A_vmax = WID * max(s2max, 0.0)
    Q_vmax = _manning_q_np(A_vmax, WID, SS1, SS2, MAN, SL)
    max_vel = Q_vmax / max(A_vmax, EPS)

    # exact inlet nodes 0 and 1 on host (f64), matching reference BCs
    d0, d1, d2 = (float(depth[0]), float(depth[1]), float(depth[2]))

    def _surf(d):
        t = min(a0 + a1 * d, d + b0)
        return max(d + b0 - t, 0.0)

    A0, A1, A2 = (WID * _surf(d0), WID * _surf(d1), WID * _surf(d2))
    # slope[0] = 0; slope[1] = minmod(A1-A0, A2-A1)
    x, y = A1 - A0, A2 - A1
    mm1 = min(max(y, min(x, 0.0)), max(x, 0.0))
    Af0 = A0
    Af1 = A1 + 0.5 * mm1
    q0 = _manning_q_np(Af0, WID, SS1, SS2, MAN, SL)
    q1 = _manning_q_np(Af1, WID, SS1, SS2, MAN, SL)
    An0 = max(A0 - r * (q0 - 0.0), 0.0)
    An1 = max(A1 - r * (q1 - q0), 0.0)
    for An in (An0, An1):
        Q = _manning_q_np(An, WID, SS1, SS2, MAN, SL)
        max_vel = max(max_vel, Q / max(An, EPS))

    max_cfl = max_vel * dt / dx

    # outlet discharge from the device's last updated state
    sfl = max(float(outs[NCORES - 1][P - 1, 4 * nch]), 0.0)
    A_last = WID * sfl
    outflow_q = _manning_q_np(A_last, WID, SS1, SS2, MAN, SL)

    out = np.array([outflow_q, infil_rate, infil_depth, max_cfl], np.float32)
    if _return_results:
        return out, res
    return out
